# revision 2
# baseline (speedup 1.0000x reference)
"""Trainium2 Bass kernel for nn_CombineConcat (pairwise broadcast+concat).

reference semantics (per batch b):
  out[b, i*N + j, 0:D]   = x1[b, i, :]
  out[b, i*N + j, D:2*D] = x2[b, j, :]

Shapes (hardcoded): x1, x2 = [16, 128, 256] f32 -> out = [16, 16384, 512] f32.

Strategy: data-parallel over batch, 2 batches per core on 8 cores. Output
write-bound (64 MB/core). j-major SBUF layout: each ring slot is
[128, 8*512] f32 where partition p holds 8 consecutive output rows
(16 KB contiguous per partition) of a 1024-row group g:
  row g*1024 + 8p + r  =  [x1[8g + p//16] | x2[8*(p%16) + r]]
Both halves are materialized by one-hot selector matmuls on the
otherwise-idle PE (x1: K=24 replicates row 8g+u to partition group u per
2 MB output group; x2: K=48 replicates x2[b] across the 8 partition
groups once per batch), fanned into slots by DVE/ACT broadcast-read
copies. Inputs are pre-split on the host into 3 bf16 terms (hi/lo1/lo2)
stacked on K so one matmul sums them; every partial sum is exactly
representable, so the output is bit-exact f32.

Ramp engineering (the kernel is a ~185 us saturated write; all slack is
at the ends): the critical first loads are one small DMA (selectors +
batch-0 g=0 x1 rhs) plus four independent x2 column-chunk DMAs spread
over both HWDGE queues; px2 lives in four single-bank PSUM tiles so the
four x2-replication matmuls don't false-WAR-serialize against the chunk
copies; slot-0's x1 fanout runs on ACT concurrently with the px2 chunk
copies on DVE, both in quarter chunks; and group 0's output DMA is
emitted as four column-quarter DMAs each gated only on its own quarter's
fills. The slot mapping rotates by 4 between batches so batch-1 x2
refills overlap batch-0's tail; the last two groups' DMAs are split by
column across both HWDGE queues to flatten the drain.
"""

import numpy as np

_B, _N, _D = 16, 128, 256
_NCORES = 8
_BPC = _B // _NCORES  # batches per core
_ROWS = 8  # output rows per partition per slot
_GRP = _N * _ROWS  # dram rows per output dma (1024)
_NGRP = _N * _N // _GRP  # groups per batch (16)
_NSLOTS = 6

_NC_CACHE = {}


def _build_nc():
    import concourse.bacc as bacc
    import concourse.mybir as mybir
    from concourse.tile import TileContext
    from concourse.bass import MemorySpace

    f32 = mybir.dt.float32
    bf16 = mybir.dt.bfloat16
    bpc, n, d = _BPC, _N, _D
    W = _ROWS * 2 * d  # 4096 f32 per partition per slot
    nq = n // _ROWS  # 16 partition-groups / x2 rows per group

    nc = bacc.Bacc("TRN2", target_bir_lowering=False, enable_partition_id=False)
    # host-prearranged inputs (see _run). x1/x2 are split into 3 exact bf16
    # terms (hi/lo1/lo2) stacked on the matmul K (partition) dim, so one
    # matmul sums all three terms (every partial sum is exactly
    # representable, so the result is bit-exact f32):
    #   x1all[b, 8j+u, g*256+c] = term_j(x1[b, 8g+u, c])      K=24
    #   x2all[b, 16j+q, r*256+c] = term_j(x2[b, 8q+r, c])     K=48
    # selall cols 0:128 = sel2_3 [48,128]: [16j+q, p]=1 iff p%16==q
    #        cols 128:256 rows 0:24 = sel1_3 [24,128]: [8j+u, p]=1 iff p//16==u
    #        cols 256:512 rows 0:24 = batch-0 g=0 x1 rhs (rides the first DMA)
    x1all = nc.dram_tensor("x1all", [bpc, 3 * _ROWS, _NGRP * d], bf16, kind="ExternalInput")
    x2all = nc.dram_tensor("x2all", [bpc, 3 * nq, _ROWS * d], bf16, kind="ExternalInput")
    selall = nc.dram_tensor("selall", [3 * nq, 2 * n + d], bf16, kind="ExternalInput")
    out = nc.dram_tensor("out", [bpc, n * n, 2 * d], f32, kind="ExternalOutput")

    with TileContext(nc) as tc:
        with (
            tc.tile_pool(name="io", bufs=1) as iop,
            tc.tile_pool(name="ring", bufs=1) as rp,
            tc.tile_pool(name="ps", bufs=1, space=MemorySpace.PSUM) as pp,
        ):
            selsb = iop.tile([3 * nq, 2 * n + d], bf16, name="selsb", tag="selsb")
            sel2ap = selsb[:, 0:n]
            sel1ap = selsb[0 : 3 * _ROWS, n : 2 * n]
            x1g0ap = selsb[0 : 3 * _ROWS, 2 * n : 2 * n + d]
            # batch-0 inputs arrive as independent small tiles so each ramp
            # matmul waits on exactly one early DMA
            x2t0c = [
                iop.tile([3 * nq, 2 * d], bf16, name=f"x2t0c_{j}", tag=f"x2t0c_{j}")
                for j in range(4)
            ]
            x1t0b = iop.tile([3 * _ROWS, 7 * d], bf16, name="x1t0b", tag="x1t0b")
            x1t0c = iop.tile([3 * _ROWS, 8 * d], bf16, name="x1t0c", tag="x1t0c")
            x1t1 = iop.tile([3 * _ROWS, _NGRP * d], bf16, name="x1t1", tag="x1t1")
            x2t1 = iop.tile([3 * nq, _ROWS * d], bf16, name="x2t1", tag="x2t1")

            # critical-path loads first: selectors + g=0 x1 rhs (sync), then
            # the four x2 chunks for batch-0's px2 matmuls (scalar queue is
            # otherwise empty so its first chunks land earliest)
            nc.sync.dma_start(out=selsb[:], in_=selall[:, :])
            nc.scalar.dma_start(out=x2t0c[0][:], in_=x2all[0][:, 0 : 2 * d])
            nc.sync.dma_start(out=x2t0c[1][:], in_=x2all[0][:, 2 * d : 4 * d])
            nc.scalar.dma_start(out=x2t0c[2][:], in_=x2all[0][:, 4 * d : 6 * d])
            nc.sync.dma_start(out=x2t0c[3][:], in_=x2all[0][:, 6 * d : 8 * d])
            nc.scalar.dma_start(out=x1t0b[:], in_=x1all[0][:, d : 8 * d])
            nc.sync.dma_start(out=x1t0c[:], in_=x1all[0][:, 8 * d : 16 * d])
            # batch-1 loads ride the ramp's idle ring time
            h1 = _NGRP * d // 2
            nc.scalar.dma_start(out=x1t1[:, 0:h1], in_=x1all[1][:, 0:h1])
            nc.sync.dma_start(out=x1t1[:, h1:], in_=x1all[1][:, h1:])
            h2 = _ROWS * d // 2
            nc.scalar.dma_start(out=x2t1[:, 0:h2], in_=x2all[1][:, 0:h2])
            nc.sync.dma_start(out=x2t1[:, h2:], in_=x2all[1][:, h2:])

            slots = [rp.tile([n, W], f32, name=f"s{k}", tag=f"s{k}") for k in range(_NSLOTS)]
            # px2 in four single-bank tiles: the four replication matmuls and
            # the four chunk copies then carry no false WAR between chunks
            px2 = [pp.tile([n, 2 * d], f32, name=f"px2_{j}", tag=f"px2_{j}") for j in range(4)]
            px1 = [pp.tile([n, 512], f32, name=f"px1_{k}", tag=f"px1_{k}") for k in range(4)]

            def x1_rhs(b, g):
                if b == 0:
                    if g == 0:
                        return x1g0ap
                    if g < 8:
                        return x1t0b[:, (g - 1) * d : g * d]
                    return x1t0c[:, (g - 8) * d : (g - 7) * d]
                return x1t1[:, g * d : (g + 1) * d]

            def x1_mm(b, g, k):
                p1 = px1[k % 4][:, 0:d]
                nc.tensor.matmul(p1, sel1ap, x1_rhs(b, g), start=True, stop=True)
                return p1

            def x2_rhs(b, j):
                if b == 0:
                    return x2t0c[j][:]
                return x2t1[:, j * 2 * d : (j + 1) * 2 * d]

            di = 0
            for b in range(bpc):
                k0 = (4 * b) % _NSLOTS
                sv0 = slots[k0][:].rearrange("p (r h c) -> p r h c", r=_ROWS, h=2)
                dst0 = out[b][0:_GRP, :].rearrange("(p r) c -> p (r c)", p=n)
                p1_first = x1_mm(b, 0, k0)
                # slot-0 fill in quarter chunks: px2 mm j -> DVE chunk copy,
                # ACT fanout chunk, then group-0's quarter output DMA gated
                # only on that quarter
                for j in range(4):
                    nc.tensor.matmul(
                        px2[j][:], sel2ap, x2_rhs(b, j), start=True, stop=True
                    )
                    nc.scalar.copy(
                        out=sv0[:, 2 * j : 2 * j + 2, 0, :],
                        in_=p1_first[:, None, :].broadcast_to((n, 2, d)),
                    )
                    nc.vector.tensor_copy(
                        out=sv0[:, 2 * j : 2 * j + 2, 1, :],
                        in_=px2[j][:].rearrange("p (r c) -> p r c", r=2),
                    )
                    cs = slice(j * 2 * 2 * d, (j + 1) * 2 * 2 * d)
                    eng = nc.sync if di % 2 == 0 else nc.scalar
                    eng.dma_start(out=dst0[:, cs], in_=slots[k0][:, cs])
                    di += 1
                for g in range(1, _NGRP):
                    # rotate slot mapping per batch so batch-1's first slots
                    # are the ones batch-0 freed earliest (refills overlap
                    # batch-0's tail instead of stalling at the boundary)
                    k = (g + 4 * b) % _NSLOTS
                    sv = slots[k][:].rearrange("p (r h c) -> p r h c", r=_ROWS, h=2)
                    if g < _NSLOTS:  # x2 half: once per slot per batch
                        for j in range(4):
                            nc.vector.tensor_copy(
                                out=sv[:, 2 * j : 2 * j + 2, 1, :],
                                in_=px2[j][:].rearrange("p (r c) -> p r c", r=2),
                            )
                    p1 = x1_mm(b, g, g)
                    ceng = nc.vector if g % 2 == 0 else nc.scalar
                    cop = ceng.tensor_copy if g % 2 == 0 else ceng.copy
                    cop(
                        out=sv[:, :, 0, :],
                        in_=p1[:, None, :].broadcast_to((n, _ROWS, d)),
                    )
                    dst = out[b][g * _GRP : (g + 1) * _GRP, :].rearrange(
                        "(p r) c -> p (r c)", p=n
                    )
                    if b == bpc - 1 and g >= _NGRP - 2:
                        # drain: split by column so each piece still spans all
                        # 16 SDMA engines (partition splits hit disjoint
                        # 8-engine sets instead)
                        hw = W // 2
                        nc.sync.dma_start(out=dst[:, 0:hw], in_=slots[k][:, 0:hw])
                        nc.scalar.dma_start(out=dst[:, hw:], in_=slots[k][:, hw:])
                    else:
                        eng = nc.sync if di % 2 == 0 else nc.scalar
                        eng.dma_start(out=dst, in_=slots[k][:])
                    di += 1
    nc.finalize()
    return nc


def _get_nc():
    if "nc" not in _NC_CACHE:
        _NC_CACHE["nc"] = _build_nc()
    return _NC_CACHE["nc"]


def _split_bf16_3(x):
    """x (f32) == hi + lo1 + lo2 exactly, each exactly bf16-representable."""
    xu = x.view(np.uint32)
    hi = (xu & np.uint32(0xFFFF0000)).view(np.float32)
    r1 = x - hi
    r1u = r1.view(np.uint32)
    lo1 = (r1u & np.uint32(0xFFFF0000)).view(np.float32)
    lo2 = r1 - lo1
    import ml_dtypes

    return (
        hi.astype(ml_dtypes.bfloat16),
        lo1.astype(ml_dtypes.bfloat16),
        lo2.astype(ml_dtypes.bfloat16),
    )


def _run(x1, x2, trace=False):
    """Run the kernel on 8 cores; returns (output, BassKernelResults)."""
    from concourse.bass_utils import run_bass_kernel_spmd

    nc = _get_nc()
    x1 = np.ascontiguousarray(np.asarray(x1, dtype=np.float32))
    x2 = np.ascontiguousarray(np.asarray(x2, dtype=np.float32))
    import ml_dtypes

    nq = _N // _ROWS
    sel1 = (
        np.arange(_N)[None, :] // 16 == np.arange(_ROWS)[:, None]
    ).astype(np.float32)
    sel2 = (
        np.arange(_N)[None, :] % 16 == np.arange(nq)[:, None]
    ).astype(np.float32)
    selbase = np.zeros((3 * nq, 2 * _N), np.float32)
    selbase[:, 0:_N] = np.tile(sel2, (3, 1))
    selbase[0 : 3 * _ROWS, _N : 2 * _N] = np.tile(sel1, (3, 1))
    selbase = selbase.astype(ml_dtypes.bfloat16)
    in_maps = []
    for c in range(_NCORES):
        x1s = x1[c * _BPC : (c + 1) * _BPC]
        x2s = x2[c * _BPC : (c + 1) * _BPC]
        # x1g[b, u, g*256+c] = x1[b, 8g+u, c]
        x1g = np.ascontiguousarray(
            x1s.reshape(_BPC, _NGRP, _ROWS, _D).transpose(0, 2, 1, 3)
        ).reshape(_BPC, _ROWS, _NGRP * _D)
        x1all = np.concatenate(_split_bf16_3(x1g), axis=1)  # [bpc, 24, 4096]
        x2all = np.concatenate(
            _split_bf16_3(x2s.reshape(_BPC, nq, _ROWS * _D)), axis=1
        )  # [bpc, 48, 2048]
        x1g0 = np.zeros((3 * nq, _D), ml_dtypes.bfloat16)
        x1g0[0 : 3 * _ROWS] = x1all[0][:, 0:_D]
        selall = np.concatenate([selbase, x1g0], axis=1)
        in_maps.append(
            {
                "x1all": np.ascontiguousarray(x1all),
                "x2all": np.ascontiguousarray(x2all),
                "selall": np.ascontiguousarray(selall),
            }
        )
    res = run_bass_kernel_spmd(
        nc, in_maps, core_ids=list(range(_NCORES)), trace=trace
    )
    out = np.concatenate([r["out"] for r in res.results], axis=0)
    return out, res


def kernel(x1, x2):
    out, _ = _run(x1, x2, trace=False)
    return out


# revision 3
# speedup vs baseline: 1.2223x; 1.2223x over previous
"""Trainium2 Bass kernel for nn_CombineConcat (pairwise broadcast+concat).

reference semantics (per batch b):
  out[b, i*N + j, 0:D]   = x1[b, i, :]
  out[b, i*N + j, D:2*D] = x2[b, j, :]

Shapes (hardcoded): x1, x2 = [16, 128, 256] f32 -> out = [16, 16384, 512] f32.

Strategy: data-parallel over batch, 2 batches per core on 8 cores. Output
write-bound (64 MB/core). j-major SBUF layout: each ring slot is
[128, 8*512] f32 where partition p holds 8 consecutive output rows
(16 KB contiguous per partition) of a 1024-row group g:
  row g*1024 + 8p + r  =  [x1[8g + p//16] | x2[8*(p%16) + r]]
Both halves are materialized by one-hot selector matmuls on the
otherwise-idle PE (x1: K=24 replicates row 8g+u to partition group u per
2 MB output group; x2: K=48 replicates x2[b] across the 8 partition
groups once per batch), fanned into slots by DVE/ACT broadcast-read
copies. Inputs are pre-split on the host into 3 bf16 terms (hi/lo1/lo2)
stacked on K so one matmul sums them; every partial sum is exactly
representable, so the output is bit-exact f32.

Ramp engineering (the kernel is a ~185 us saturated write; all slack is
at the ends): the critical first loads are one small DMA (selectors +
batch-0 g=0 x1 rhs) plus four independent x2 column-chunk DMAs spread
over both HWDGE queues; px2 lives in four single-bank PSUM tiles so the
four x2-replication matmuls don't false-WAR-serialize against the chunk
copies; slot-0's x1 fanout runs on ACT concurrently with the px2 chunk
copies on DVE, both in quarter chunks; and group 0's output DMA is
emitted as four column-quarter DMAs each gated only on its own quarter's
fills. The slot mapping rotates by 4 between batches so batch-1 x2
refills overlap batch-0's tail; the last two groups' DMAs are split by
column across both HWDGE queues to flatten the drain.
"""

import numpy as np

_B, _N, _D = 16, 128, 256
_NCORES = 8
_BPC = _B // _NCORES  # batches per core
_ROWS = 8  # output rows per partition per slot
_GRP = _N * _ROWS  # dram rows per output dma (1024)
_NGRP = _N * _N // _GRP  # groups per batch (16)
_NSLOTS = 6

_NC_CACHE = {}


def _build_nc():
    import concourse.bacc as bacc
    import concourse.mybir as mybir
    from concourse.tile import TileContext
    from concourse.bass import MemorySpace

    f32 = mybir.dt.float32
    bf16 = mybir.dt.bfloat16
    bpc, n, d = _BPC, _N, _D
    W = _ROWS * 2 * d  # 4096 f32 per partition per slot
    nq = n // _ROWS  # 16 partition-groups / x2 rows per group

    nc = bacc.Bacc("TRN2", target_bir_lowering=False, enable_partition_id=False)
    # host-prearranged inputs (see _run). x1/x2 are split into 3 exact bf16
    # terms (hi/lo1/lo2) stacked on the matmul K (partition) dim, so one
    # matmul sums all three terms (every partial sum is exactly
    # representable, so the result is bit-exact f32):
    #   x1all[b, 8j+u, g*256+c] = term_j(x1[b, 8g+u, c])      K=24
    #   x2all[b, 16j+q, r*256+c] = term_j(x2[b, 8q+r, c])     K=48
    # selall cols 0:128 = sel2_3 [48,128]: [16j+q, p]=1 iff p%16==q
    #        cols 128:256 rows 0:24 = sel1_3 [24,128]: [8j+u, p]=1 iff p//16==u
    #        cols 256:512 rows 0:24 = batch-0 g=0 x1 rhs (rides the first DMA)
    x1all = nc.dram_tensor("x1all", [bpc, 3 * _ROWS, _NGRP * d], bf16, kind="ExternalInput")
    x2all = nc.dram_tensor("x2all", [bpc, 3 * nq, _ROWS * d], bf16, kind="ExternalInput")
    selall = nc.dram_tensor("selall", [3 * nq, 2 * n + d], bf16, kind="ExternalInput")
    out = nc.dram_tensor("out", [bpc, n * n, 2 * d], f32, kind="ExternalOutput")

    with TileContext(nc) as tc:
        with (
            tc.tile_pool(name="io", bufs=1) as iop,
            tc.tile_pool(name="ring", bufs=1) as rp,
            tc.tile_pool(name="ps", bufs=1, space=MemorySpace.PSUM) as pp,
        ):
            selsb = iop.tile([3 * nq, 2 * n + d], bf16, name="selsb", tag="selsb")
            sel2ap = selsb[:, 0:n]
            sel1ap = selsb[0 : 3 * _ROWS, n : 2 * n]
            x1g0ap = selsb[0 : 3 * _ROWS, 2 * n : 2 * n + d]
            # batch-0 inputs arrive as independent small tiles so each ramp
            # matmul waits on exactly one early DMA
            x2t0c = [
                iop.tile([3 * nq, 2 * d], bf16, name=f"x2t0c_{j}", tag=f"x2t0c_{j}")
                for j in range(4)
            ]
            x1t0b = iop.tile([3 * _ROWS, 7 * d], bf16, name="x1t0b", tag="x1t0b")
            x1t0c = iop.tile([3 * _ROWS, 8 * d], bf16, name="x1t0c", tag="x1t0c")
            x1t1 = iop.tile([3 * _ROWS, _NGRP * d], bf16, name="x1t1", tag="x1t1")
            x2t1 = iop.tile([3 * nq, _ROWS * d], bf16, name="x2t1", tag="x2t1")

            # critical-path loads first: selectors + g=0 x1 rhs (sync), then
            # the four x2 chunks for batch-0's px2 matmuls (scalar queue is
            # otherwise empty so its first chunks land earliest)
            nc.sync.dma_start(out=selsb[:], in_=selall[:, :])
            nc.scalar.dma_start(out=x2t0c[0][:], in_=x2all[0][:, 0 : 2 * d])
            nc.sync.dma_start(out=x2t0c[1][:], in_=x2all[0][:, 2 * d : 4 * d])
            nc.scalar.dma_start(out=x2t0c[2][:], in_=x2all[0][:, 4 * d : 6 * d])
            nc.sync.dma_start(out=x2t0c[3][:], in_=x2all[0][:, 6 * d : 8 * d])
            nc.scalar.dma_start(out=x1t0b[:], in_=x1all[0][:, d : 8 * d])
            nc.sync.dma_start(out=x1t0c[:], in_=x1all[0][:, 8 * d : 16 * d])
            # batch-1 loads ride the ramp's idle ring time
            h1 = _NGRP * d // 2
            nc.scalar.dma_start(out=x1t1[:, 0:h1], in_=x1all[1][:, 0:h1])
            nc.sync.dma_start(out=x1t1[:, h1:], in_=x1all[1][:, h1:])
            h2 = _ROWS * d // 2
            nc.scalar.dma_start(out=x2t1[:, 0:h2], in_=x2all[1][:, 0:h2])
            nc.sync.dma_start(out=x2t1[:, h2:], in_=x2all[1][:, h2:])

            # pad so each slot's per-partition base lands at the same SBUF
            # residue as the fast baseline layout (mod-1024 = 128); the
            # misaligned layout measurably slows the 16 KB output descriptors
            rp.tile([n, 128], f32, name="pad", tag="pad")
            slots = [rp.tile([n, W], f32, name=f"s{k}", tag=f"s{k}") for k in range(_NSLOTS)]
            # px2 in four single-bank tiles: the four replication matmuls and
            # the four chunk copies then carry no false WAR between chunks
            px2 = [pp.tile([n, 2 * d], f32, name=f"px2_{j}", tag=f"px2_{j}") for j in range(4)]
            px1 = [pp.tile([n, 512], f32, name=f"px1_{k}", tag=f"px1_{k}") for k in range(4)]

            def x1_rhs(b, g):
                if b == 0:
                    if g == 0:
                        return x1g0ap
                    if g < 8:
                        return x1t0b[:, (g - 1) * d : g * d]
                    return x1t0c[:, (g - 8) * d : (g - 7) * d]
                return x1t1[:, g * d : (g + 1) * d]

            def x1_mm(b, g, k):
                p1 = px1[k % 4][:, 0:d]
                nc.tensor.matmul(p1, sel1ap, x1_rhs(b, g), start=True, stop=True)
                return p1

            def x2_rhs(b, j):
                if b == 0:
                    return x2t0c[j][:]
                return x2t1[:, j * 2 * d : (j + 1) * 2 * d]

            di = 0
            for b in range(bpc):
                k0 = (4 * b) % _NSLOTS
                sv0 = slots[k0][:].rearrange("p (r h c) -> p r h c", r=_ROWS, h=2)
                dst0 = out[b][0:_GRP, :].rearrange("(p r) c -> p (r c)", p=n)
                p1_first = x1_mm(b, 0, k0)
                # slot-0 fill in quarter chunks: px2 mm j -> DVE chunk copy,
                # ACT fanout chunk, then group-0's quarter output DMA gated
                # only on that quarter
                for j in range(4):
                    nc.tensor.matmul(
                        px2[j][:], sel2ap, x2_rhs(b, j), start=True, stop=True
                    )
                    nc.scalar.copy(
                        out=sv0[:, 2 * j : 2 * j + 2, 0, :],
                        in_=p1_first[:, None, :].broadcast_to((n, 2, d)),
                    )
                    nc.vector.tensor_copy(
                        out=sv0[:, 2 * j : 2 * j + 2, 1, :],
                        in_=px2[j][:].rearrange("p (r c) -> p r c", r=2),
                    )
                    cs = slice(j * 2 * 2 * d, (j + 1) * 2 * 2 * d)
                    eng = nc.sync if di % 2 == 0 else nc.scalar
                    eng.dma_start(out=dst0[:, cs], in_=slots[k0][:, cs])
                    di += 1
                for g in range(1, _NGRP):
                    # rotate slot mapping per batch so batch-1's first slots
                    # are the ones batch-0 freed earliest (refills overlap
                    # batch-0's tail instead of stalling at the boundary)
                    k = (g + 4 * b) % _NSLOTS
                    sv = slots[k][:].rearrange("p (r h c) -> p r h c", r=_ROWS, h=2)
                    if g < _NSLOTS:  # x2 half: once per slot per batch
                        for j in range(4):
                            nc.vector.tensor_copy(
                                out=sv[:, 2 * j : 2 * j + 2, 1, :],
                                in_=px2[j][:].rearrange("p (r c) -> p r c", r=2),
                            )
                    p1 = x1_mm(b, g, g)
                    ceng = nc.vector if g % 2 == 0 else nc.scalar
                    cop = ceng.tensor_copy if g % 2 == 0 else ceng.copy
                    cop(
                        out=sv[:, :, 0, :],
                        in_=p1[:, None, :].broadcast_to((n, _ROWS, d)),
                    )
                    dst = out[b][g * _GRP : (g + 1) * _GRP, :].rearrange(
                        "(p r) c -> p (r c)", p=n
                    )
                    if b == bpc - 1 and g >= _NGRP - 2:
                        # drain: split by column so each piece still spans all
                        # 16 SDMA engines (partition splits hit disjoint
                        # 8-engine sets instead)
                        hw = W // 2
                        nc.sync.dma_start(out=dst[:, 0:hw], in_=slots[k][:, 0:hw])
                        nc.scalar.dma_start(out=dst[:, hw:], in_=slots[k][:, hw:])
                    else:
                        eng = nc.sync if di % 2 == 0 else nc.scalar
                        eng.dma_start(out=dst, in_=slots[k][:])
                    di += 1
    nc.finalize()
    return nc


def _get_nc():
    if "nc" not in _NC_CACHE:
        _NC_CACHE["nc"] = _build_nc()
    return _NC_CACHE["nc"]


def _split_bf16_3(x):
    """x (f32) == hi + lo1 + lo2 exactly, each exactly bf16-representable."""
    xu = x.view(np.uint32)
    hi = (xu & np.uint32(0xFFFF0000)).view(np.float32)
    r1 = x - hi
    r1u = r1.view(np.uint32)
    lo1 = (r1u & np.uint32(0xFFFF0000)).view(np.float32)
    lo2 = r1 - lo1
    import ml_dtypes

    return (
        hi.astype(ml_dtypes.bfloat16),
        lo1.astype(ml_dtypes.bfloat16),
        lo2.astype(ml_dtypes.bfloat16),
    )


def _run(x1, x2, trace=False):
    """Run the kernel on 8 cores; returns (output, BassKernelResults)."""
    from concourse.bass_utils import run_bass_kernel_spmd

    nc = _get_nc()
    x1 = np.ascontiguousarray(np.asarray(x1, dtype=np.float32))
    x2 = np.ascontiguousarray(np.asarray(x2, dtype=np.float32))
    import ml_dtypes

    nq = _N // _ROWS
    sel1 = (
        np.arange(_N)[None, :] // 16 == np.arange(_ROWS)[:, None]
    ).astype(np.float32)
    sel2 = (
        np.arange(_N)[None, :] % 16 == np.arange(nq)[:, None]
    ).astype(np.float32)
    selbase = np.zeros((3 * nq, 2 * _N), np.float32)
    selbase[:, 0:_N] = np.tile(sel2, (3, 1))
    selbase[0 : 3 * _ROWS, _N : 2 * _N] = np.tile(sel1, (3, 1))
    selbase = selbase.astype(ml_dtypes.bfloat16)
    in_maps = []
    for c in range(_NCORES):
        x1s = x1[c * _BPC : (c + 1) * _BPC]
        x2s = x2[c * _BPC : (c + 1) * _BPC]
        # x1g[b, u, g*256+c] = x1[b, 8g+u, c]
        x1g = np.ascontiguousarray(
            x1s.reshape(_BPC, _NGRP, _ROWS, _D).transpose(0, 2, 1, 3)
        ).reshape(_BPC, _ROWS, _NGRP * _D)
        x1all = np.concatenate(_split_bf16_3(x1g), axis=1)  # [bpc, 24, 4096]
        x2all = np.concatenate(
            _split_bf16_3(x2s.reshape(_BPC, nq, _ROWS * _D)), axis=1
        )  # [bpc, 48, 2048]
        x1g0 = np.zeros((3 * nq, _D), ml_dtypes.bfloat16)
        x1g0[0 : 3 * _ROWS] = x1all[0][:, 0:_D]
        selall = np.concatenate([selbase, x1g0], axis=1)
        in_maps.append(
            {
                "x1all": np.ascontiguousarray(x1all),
                "x2all": np.ascontiguousarray(x2all),
                "selall": np.ascontiguousarray(selall),
            }
        )
    res = run_bass_kernel_spmd(
        nc, in_maps, core_ids=list(range(_NCORES)), trace=trace
    )
    out = np.concatenate([r["out"] for r in res.results], axis=0)
    return out, res


def kernel(x1, x2):
    out, _ = _run(x1, x2, trace=False)
    return out
